# revision 3
# baseline (speedup 1.0000x reference)
"""Trainium2 Bass kernel for single-head attention model.

Reference computation (B=4, S=2048, E=1024, fp32):
    q = query @ Wq + bq;  k = key @ Wk + bk;  v = value @ Wv + bv
    scores = (q @ k^T) / sqrt(E)
    out = softmax(scores, axis=-1) @ v

Sharding: 8 cores; core c handles batch b = c // 2, query-row half
h = c % 2 (1024 q-rows). K/V projections for the full batch are
computed redundantly on both cores of a pair (no collectives).

v2 layout strategy: all inputs/weights converted to bf16 on host
(same 1 cycle/row PE rate as f32r, half the DMA bytes and SBUF
footprint).  Phase order K-proj -> Q-proj -> V-proj -> attention so
each phase's DMA prefetch hides under the previous phase's compute:
  - KT[e, s_k]  = Wk^T xkT   (lhsT = Wk natural layout)
  - QT[e, s_q]  = Wq^T xqT
  - V[s_k, e]   = xvT^T Wv   (lhsT = xvT slices)
  - scoresT[s_k, s_q_blk] = KT^T_slices @ QT   (transposed scores)
  - expT = exp(scoresT / 32) in bf16 -- no max subtraction; scores O(1)
  - out_unnorm[s_q, e] = expT^T @ V  (lhsT = expT slices)
  - sums[s_q] via DVE partial-sum chain over s_k tiles + tiny
    ones-matmul per 128-row group to reduce over partitions
  - out = out_unnorm * (1/sums), DMA out natural fp32
"""

import sys

sys.path.insert(0, "/opt/trn_rl_repo")

from contextlib import ExitStack

import numpy as np
import ml_dtypes

import concourse.bass as bass
import concourse.mybir as mybir
import concourse.tile as tile
from concourse import bacc, bass_utils

F32R = mybir.dt.float32r
F32 = mybir.dt.float32
BF = mybir.dt.bfloat16
AF = mybir.ActivationFunctionType
NPBF = ml_dtypes.bfloat16

B, S, E = 4, 2048, 1024
N_CORES = 8
SQ = S // 2          # q rows per core
SK = S               # kv rows per core
BQ = 512             # s_q block width in phase D
NBLK = SQ // BQ      # 2 blocks
EK = E // 128        # 8 contraction tiles over e
MK = SK // 128       # 16 s_k tiles
INV_SCALE = 1.0 / float(np.sqrt(E))

_cached = {}


def _build():
    nc = bacc.Bacc("TRN2", target_bir_lowering=False, debug=False,
                   num_devices=N_CORES)

    xqT = nc.dram_tensor("xqT", [E, SQ], BF, kind="ExternalInput").ap()
    xkT = nc.dram_tensor("xkT", [E, SK], BF, kind="ExternalInput").ap()
    xvT = nc.dram_tensor("xvT", [E, SK], BF, kind="ExternalInput").ap()
    wq = nc.dram_tensor("wq", [E, E], BF, kind="ExternalInput").ap()
    wk = nc.dram_tensor("wk", [E, E], BF, kind="ExternalInput").ap()
    wv = nc.dram_tensor("wv", [E, E], BF, kind="ExternalInput").ap()
    # biases pre-arranged on host: b_sb[p, t] = b[t*128 + p]
    bqh = nc.dram_tensor("bqh", [128, EK], F32, kind="ExternalInput").ap()
    bkh = nc.dram_tensor("bkh", [128, EK], F32, kind="ExternalInput").ap()
    bvh = nc.dram_tensor("bvh", [1, E], F32, kind="ExternalInput").ap()
    ones_in = nc.dram_tensor("ones_in", [128, 2], F32R, kind="ExternalInput").ap()
    out = nc.dram_tensor("out", [SQ, E], F32, kind="ExternalOutput").ap()

    with tile.TileContext(nc) as tc, ExitStack() as top:
        # ---- long-lived pools ----
        consts = top.enter_context(tc.tile_pool(name="consts", bufs=1))
        ktpool = top.enter_context(tc.tile_pool(name="ktpool", bufs=1))
        qtpool = top.enter_context(tc.tile_pool(name="qtpool", bufs=1))
        vpool = top.enter_context(tc.tile_pool(name="vpool", bufs=1))
        wkp = top.enter_context(tc.tile_pool(name="wkp", bufs=1))
        wvp = top.enter_context(tc.tile_pool(name="wvp", bufs=1))
        xqp = top.enter_context(tc.tile_pool(name="xqp", bufs=1))
        expp = top.enter_context(tc.tile_pool(name="expp", bufs=1))

        ones_t = consts.tile([128, 2], F32R)
        nc.sync.dma_start(ones_t[:], ones_in)
        bq_t = consts.tile([128, EK], F32)
        nc.sync.dma_start(bq_t[:], bqh)
        bk_t = consts.tile([128, EK], F32)
        nc.sync.dma_start(bk_t[:], bkh)
        bv_row = consts.tile([1, E], F32)
        nc.sync.dma_start(bv_row[:], bvh)

        kt_tiles = [ktpool.tile([128, SK], BF, tag=f"kt{m}", name=f"kt{m}")
                    for m in range(EK)]
        qt_tiles = [qtpool.tile([128, SQ], BF, tag=f"qt{m}", name=f"qt{m}")
                    for m in range(EK)]
        v_tiles = [vpool.tile([128, E], BF, tag=f"v{m}", name=f"v{m}")
                   for m in range(MK)]
        wk_tiles = [wkp.tile([128, E], BF, tag=f"wk{k}", name=f"wk{k}")
                    for k in range(EK)]
        wv_tiles = [wvp.tile([128, E], BF, tag=f"wv{k}", name=f"wv{k}")
                    for k in range(EK)]
        xq_tiles = [xqp.tile([128, SQ], BF, tag=f"xq{k}", name=f"xq{k}")
                    for k in range(EK)]
        exp_tiles = [expp.tile([128, BQ], BF, tag=f"exp{m}", name=f"exp{m}")
                     for m in range(MK)]

        # ======== phase B: KT = Wk^T @ xkT + bk  (m-sliced wk DMA) ========
        with tc.tile_pool(name="xkblk", bufs=2) as xkp, \
             tc.tile_pool(name="psB", bufs=8, space="PSUM") as psB:
            # critical-path DMA order: wk m=0 slices, first xk block, rest
            for k in range(EK):
                nc.sync.dma_start(wk_tiles[k][:, 0:128], wk[k * 128:(k + 1) * 128, 0:128])
            xk_first = [xkp.tile([128, 512], BF, tag=f"xkb{k}", name=f"xkb0_{k}")
                        for k in range(EK)]
            for k in range(EK):
                nc.sync.dma_start(xk_first[k][:], xkT[k * 128:(k + 1) * 128, 0:512])
            for m in range(1, EK):
                for k in range(EK):
                    nc.sync.dma_start(
                        wk_tiles[k][:, m * 128:(m + 1) * 128],
                        wk[k * 128:(k + 1) * 128, m * 128:(m + 1) * 128])
            for nb in range(SK // 512):
                if nb == 0:
                    xk_blk = xk_first
                else:
                    xk_blk = [xkp.tile([128, 512], BF, tag=f"xkb{k}", name=f"xkb{nb}_{k}")
                              for k in range(EK)]
                    for k in range(EK):
                        nc.sync.dma_start(
                            xk_blk[k][:],
                            xkT[k * 128:(k + 1) * 128, nb * 512:(nb + 1) * 512])
                for m in range(EK):
                    ps = psB.tile([128, 512], F32, tag="psB")
                    for k in range(EK):
                        nc.tensor.matmul(
                            ps[:],
                            wk_tiles[k][:, m * 128:(m + 1) * 128],
                            xk_blk[k][:],
                            start=(k == 0), stop=(k == EK - 1))
                    nc.vector.tensor_scalar_add(
                        kt_tiles[m][:, nb * 512:(nb + 1) * 512],
                        ps[:], bk_t[:, m:m + 1])

        # ======== phase C: QT = Wq^T @ xqT + bq ========
        with tc.tile_pool(name="wqblk", bufs=2) as wqp, \
             tc.tile_pool(name="psC", bufs=8, space="PSUM") as psC:
            for k in range(EK):
                nc.sync.dma_start(xq_tiles[k][:], xqT[k * 128:(k + 1) * 128, :])
            for m in range(EK):
                wq_blk = [wqp.tile([128, 128], BF, tag=f"wqb{k}", name=f"wqb{m}_{k}")
                          for k in range(EK)]
                for k in range(EK):
                    nc.sync.dma_start(
                        wq_blk[k][:],
                        wq[k * 128:(k + 1) * 128, m * 128:(m + 1) * 128])
                for n in range(SQ // 512):
                    ps = psC.tile([128, 512], F32, tag="psC")
                    for k in range(EK):
                        nc.tensor.matmul(
                            ps[:], wq_blk[k][:],
                            xq_tiles[k][:, n * 512:(n + 1) * 512],
                            start=(k == 0), stop=(k == EK - 1))
                    nc.vector.tensor_scalar_add(
                        qt_tiles[m][:, n * 512:(n + 1) * 512],
                        ps[:], bq_t[:, m:m + 1])

        # ======== phase A: V = xvT^T @ Wv + bv ========
        with tc.tile_pool(name="xvblk", bufs=2) as xvp, \
             tc.tile_pool(name="bvp", bufs=1) as bvp, \
             tc.tile_pool(name="psA", bufs=2, space="PSUM") as psA:
            bv_bc = bvp.tile([128, E], F32)
            nc.gpsimd.partition_broadcast(bv_bc[:], bv_row[:])
            for n in range(E // 512):
                for k in range(EK):
                    nc.sync.dma_start(
                        wv_tiles[k][:, n * 512:(n + 1) * 512],
                        wv[k * 128:(k + 1) * 128, n * 512:(n + 1) * 512])
            for mb in range(SK // 512):
                xv_blk = [xvp.tile([128, 512], BF, tag=f"xvb{k}", name=f"xvb{mb}_{k}")
                          for k in range(EK)]
                for k in range(EK):
                    nc.sync.dma_start(
                        xv_blk[k][:],
                        xvT[k * 128:(k + 1) * 128, mb * 512:(mb + 1) * 512])
                for n in range(E // 512):
                    pss = [psA.tile([128, 512], F32, tag=f"psA{i}", name=f"psA_{mb}_{n}_{i}")
                           for i in range(4)]
                    for k in range(EK):
                        for i in range(4):
                            nc.tensor.matmul(
                                pss[i][:],
                                xv_blk[k][:, i * 128:(i + 1) * 128],
                                wv_tiles[k][:, n * 512:(n + 1) * 512],
                                start=(k == 0), stop=(k == EK - 1))
                    for i in range(4):
                        m = mb * 4 + i
                        nc.vector.tensor_add(
                            v_tiles[m][:, n * 512:(n + 1) * 512],
                            pss[i][:],
                            bv_bc[:, n * 512:(n + 1) * 512])

        # ======== phase D: attention, blocked over s_q ========
        with tc.tile_pool(name="partp", bufs=2) as partp, \
             tc.tile_pool(name="outp", bufs=2) as outp, \
             tc.tile_pool(name="sumsp", bufs=4) as sumsp, \
             tc.tile_pool(name="psS", bufs=4, space="PSUM") as psS, \
             tc.tile_pool(name="psO", bufs=3, space="PSUM") as psO, \
             tc.tile_pool(name="psSum", bufs=1, space="PSUM") as psSum:
            for blk in range(NBLK):
                q0 = blk * BQ
                # scoresT[s_k, blk] = KT^T @ QT_blk ; exp -> expT tiles (bf16)
                for m in range(MK):
                    ps = psS.tile([128, BQ], F32, tag="psS")
                    for k in range(EK):
                        nc.tensor.matmul(
                            ps[:],
                            kt_tiles[k][:, m * 128:(m + 1) * 128],
                            qt_tiles[k][:, q0:q0 + BQ],
                            start=(k == 0), stop=(k == EK - 1))
                    nc.scalar.activation(exp_tiles[m][:], ps[:], AF.Exp,
                                         scale=INV_SCALE)

                # partial sums over s_k tiles (DVE chain), last write f32r
                part = partp.tile([128, BQ], F32, tag="part")
                nc.vector.tensor_add(part[:], exp_tiles[0][:], exp_tiles[1][:])
                for m in range(2, MK - 1):
                    nc.vector.tensor_add(part[:], part[:], exp_tiles[m][:])
                part_r = partp.tile([128, BQ], F32R, tag="part_r")
                nc.vector.tensor_add(part_r[:], part[:], exp_tiles[MK - 1][:])

                # out_unnorm[s_q, e] = expT^T @ V ; sums via ones-matmul
                for mi in range(BQ // 128):
                    pssum = psSum.tile([128, 2], F32, tag="pssum")
                    nc.tensor.matmul(
                        pssum[:],
                        part_r[:, mi * 128:(mi + 1) * 128],
                        ones_t[:], start=True, stop=True)
                    recip = sumsp.tile([128, 1], F32, tag="recip")
                    nc.vector.reciprocal(recip[:], pssum[:, 0:1])

                    ot = outp.tile([128, E], F32, tag="ot")
                    for n in range(E // 512):
                        pso = psO.tile([128, 512], F32, tag="psO")
                        for m in range(MK):
                            nc.tensor.matmul(
                                pso[:],
                                exp_tiles[m][:, mi * 128:(mi + 1) * 128],
                                v_tiles[m][:, n * 512:(n + 1) * 512],
                                start=(m == 0), stop=(m == MK - 1))
                        nc.vector.tensor_scalar_mul(
                            ot[:, n * 512:(n + 1) * 512], pso[:], recip[:])
                    nc.sync.dma_start(
                        out[q0 + mi * 128:q0 + (mi + 1) * 128, :], ot[:])

    nc.compile()
    return nc


def _get_nc():
    if "nc" not in _cached:
        _cached["nc"] = _build()
    return _cached["nc"]


def kernel(query, key, value, Wq, bq, Wk, bk, Wv, bv, **kw):
    query = np.asarray(query, dtype=np.float32)
    key = np.asarray(key, dtype=np.float32)
    value = np.asarray(value, dtype=np.float32)
    Wq8 = np.ascontiguousarray(np.asarray(Wq, dtype=np.float32).astype(NPBF))
    Wk8 = np.ascontiguousarray(np.asarray(Wk, dtype=np.float32).astype(NPBF))
    Wv8 = np.ascontiguousarray(np.asarray(Wv, dtype=np.float32).astype(NPBF))
    bq = np.asarray(bq, dtype=np.float32)
    bk = np.asarray(bk, dtype=np.float32)
    bv = np.asarray(bv, dtype=np.float32)

    bq_h = np.ascontiguousarray(bq.reshape(EK, 128).T)
    bk_h = np.ascontiguousarray(bk.reshape(EK, 128).T)
    bv_h = np.ascontiguousarray(bv.reshape(1, E))
    ones_h = np.ones((128, 2), dtype=np.float32)

    keyT = {b: np.ascontiguousarray(key[b].T.astype(NPBF)) for b in range(B)}
    valT = {b: np.ascontiguousarray(value[b].T.astype(NPBF)) for b in range(B)}

    in_maps = []
    for c in range(N_CORES):
        b, h = divmod(c, 2)
        qT = np.ascontiguousarray(query[b, h * SQ:(h + 1) * SQ, :].T.astype(NPBF))
        in_maps.append({
            "xqT": qT, "xkT": keyT[b], "xvT": valT[b],
            "wq": Wq8, "wk": Wk8, "wv": Wv8,
            "bqh": bq_h, "bkh": bk_h, "bvh": bv_h,
            "ones_in": ones_h,
        })

    nc = _get_nc()
    res = bass_utils.run_bass_kernel_spmd(
        nc, in_maps, core_ids=list(range(N_CORES)), **kw)

    full = np.empty((B, S, E), dtype=np.float32)
    for c in range(N_CORES):
        b, h = divmod(c, 2)
        full[b, h * SQ:(h + 1) * SQ, :] = res.results[c]["out"]
    kernel.last_results = res
    return full


# revision 8
# speedup vs baseline: 1.0385x; 1.0385x over previous
"""Trainium2 Bass kernel for single-head attention model.

Reference computation (B=4, S=2048, E=1024, fp32):
    q = query @ Wq + bq;  k = key @ Wk + bk;  v = value @ Wv + bv
    scores = (q @ k^T) / sqrt(E)
    out = softmax(scores, axis=-1) @ v

Sharding: 8 cores; core c handles batch b = c // 2, query-row half
h = c % 2 (1024 q-rows). K/V projections for the full batch are
computed redundantly on both cores of a pair (no collectives).

v3 strategy:
  - all inputs/weights bf16 (same 1 cyc/row PE rate as f32r, half DMA)
  - host packs every tensor into few large partition-major blocks so
    each phase needs a handful of big DMAs (queue overhead ~0.6us/DMA)
  - phase order K-proj -> Q-proj -> V-proj -> attention; each phase's
    DMAs prefetch under the previous phase's compute
  - weights in m-major layout so the first output-column group's
    weight slices land in one contiguous DMA
  - attention: scoresT[s_k, s_q] = KT^T @ QT per 128-row tile, exp via
    ACT (bf16), per-tile running-sum adds interleaved on DVE, softmax
    normalization via tiny ones-matmul partition reduce + reciprocal,
    out_unnorm = expT^T @ V, scaled on eviction, out DMA per 512-slice
"""

import sys

sys.path.insert(0, "/opt/trn_rl_repo")

from contextlib import ExitStack

import numpy as np
import ml_dtypes

import concourse.bass as bass
import concourse.mybir as mybir
import concourse.tile as tile
from concourse import bacc, bass_utils

F32R = mybir.dt.float32r
F32 = mybir.dt.float32
BF = mybir.dt.bfloat16
AF = mybir.ActivationFunctionType
NPBF = ml_dtypes.bfloat16

B, S, E = 4, 2048, 1024
N_CORES = 8
SQ = S // 2          # q rows per core
SK = S               # kv rows per core
BQ = 512             # s_q block width in phase D
NBLK = SQ // BQ      # 2 blocks
EK = E // 128        # 8 contraction tiles over e
MK = SK // 128       # 16 s_k tiles
INV_SCALE = 1.0 / float(np.sqrt(E))

_cached = {}


def _build():
    nc = bacc.Bacc("TRN2", target_bir_lowering=False, debug=False,
                   num_devices=N_CORES)

    # xq: [128, 2(n), 8(k), 512]; xk/xv: [128, 4(nb), 8(k), 512]
    xqh = nc.dram_tensor("xqh", [128, SQ * EK], BF, kind="ExternalInput").ap()
    xkh = nc.dram_tensor("xkh", [128, SK * EK], BF, kind="ExternalInput").ap()
    xvh = nc.dram_tensor("xvh", [128, SK * EK], BF, kind="ExternalInput").ap()
    # wq/wk: [128, 8(m), 8(k), 128] m-major; wv: [128, 8(k), 1024]
    wqh = nc.dram_tensor("wqh", [128, E * EK], BF, kind="ExternalInput").ap()
    wkh = nc.dram_tensor("wkh", [128, E * EK], BF, kind="ExternalInput").ap()
    wvh = nc.dram_tensor("wvh", [128, E * EK], BF, kind="ExternalInput").ap()
    # consts: cols [2:10]=bq, [10:18]=bk  (f32)
    csth = nc.dram_tensor("csth", [128, 18], F32, kind="ExternalInput").ap()
    ones_in = nc.dram_tensor("ones_in", [128, 2], F32R, kind="ExternalInput").ap()
    # bv broadcast to all partitions (bf16)
    bvh = nc.dram_tensor("bvh", [128, E], BF, kind="ExternalInput").ap()
    out = nc.dram_tensor("out", [SQ, E], F32, kind="ExternalOutput").ap()

    with tile.TileContext(nc) as tc, ExitStack() as top:
        # ---- long-lived pools ----
        consts = top.enter_context(tc.tile_pool(name="consts", bufs=1))
        ktpool = top.enter_context(tc.tile_pool(name="ktpool", bufs=1))
        qtpool = top.enter_context(tc.tile_pool(name="qtpool", bufs=1))
        vpool = top.enter_context(tc.tile_pool(name="vpool", bufs=1))
        expp = top.enter_context(tc.tile_pool(name="expp", bufs=1))

        cst = consts.tile([128, 18], F32)
        nc.sync.dma_start(cst[:], csth)
        ones_tile = consts.tile([128, 2], F32R)
        nc.sync.dma_start(ones_tile[:], ones_in)
        ones_t = ones_tile[:]
        bq_t = cst[:, 2:10]
        bk_t = cst[:, 10:18]
        bv_bc = consts.tile([128, E], BF)
        nc.sync.dma_start(bv_bc[:], bvh)

        kt_tiles = [ktpool.tile([128, SK], BF, tag=f"kt{m}", name=f"kt{m}")
                    for m in range(EK)]
        qt_tiles = [qtpool.tile([128, SQ], BF, tag=f"qt{m}", name=f"qt{m}")
                    for m in range(EK)]
        v_tiles = [vpool.tile([128, E], BF, tag=f"v{m}", name=f"v{m}")
                   for m in range(MK)]
        exp_tiles = [expp.tile([128, BQ], BF, tag=f"exp{m}", name=f"exp{m}")
                     for m in range(MK)]

        # ======== phase B: KT = Wk^T @ xkT + bk ========
        with tc.tile_pool(name="wkp", bufs=1) as wkp, \
             tc.tile_pool(name="xkblk", bufs=2) as xkp, \
             tc.tile_pool(name="psB", bufs=8, space="PSUM") as psB:
            wk_sb = wkp.tile([128, E * EK], BF)
            # m0 weight slices first, then first xk block, then the rest
            nc.sync.dma_start(wk_sb[:, 0:E], wkh[:, 0:E])
            xk_first = xkp.tile([128, 512 * EK], BF, tag="xkb", name="xkb0")
            nc.sync.dma_start(xk_first[:], xkh[:, 0:512 * EK])
            nc.sync.dma_start(wk_sb[:, E:2 * E], wkh[:, E:2 * E])
            nc.sync.dma_start(wk_sb[:, 2 * E:4 * E], wkh[:, 2 * E:4 * E])
            nc.sync.dma_start(wk_sb[:, 4 * E:8 * E], wkh[:, 4 * E:8 * E])
            for nb in range(SK // 512):
                if nb == 0:
                    xk_blk = xk_first
                else:
                    xk_blk = xkp.tile([128, 512 * EK], BF, tag="xkb",
                                      name=f"xkb{nb}")
                    nc.sync.dma_start(
                        xk_blk[:], xkh[:, nb * 512 * EK:(nb + 1) * 512 * EK])
                for m in range(EK):
                    ps = psB.tile([128, 512], F32, tag="psB")
                    for k in range(EK):
                        nc.tensor.matmul(
                            ps[:],
                            wk_sb[:, m * E + k * 128:m * E + (k + 1) * 128],
                            xk_blk[:, k * 512:(k + 1) * 512],
                            start=(k == 0), stop=(k == EK - 1))
                    nc.vector.tensor_scalar_add(
                        kt_tiles[m][:, nb * 512:(nb + 1) * 512],
                        ps[:], bk_t[:, m:m + 1])

        # ======== phase C: QT = Wq^T @ xqT + bq ========
        with tc.tile_pool(name="wqp", bufs=1) as wqp, \
             tc.tile_pool(name="xqp", bufs=1) as xqp, \
             tc.tile_pool(name="psC", bufs=8, space="PSUM") as psC:
            wq_sb = wqp.tile([128, E * EK], BF)
            xq_sb = xqp.tile([128, SQ * EK], BF)
            nc.sync.dma_start(wq_sb[:, 0:2 * E], wqh[:, 0:2 * E])
            nc.sync.dma_start(xq_sb[:, 0:SQ * EK // 2], xqh[:, 0:SQ * EK // 2])
            nc.sync.dma_start(wq_sb[:, 2 * E:8 * E], wqh[:, 2 * E:8 * E])
            nc.sync.dma_start(xq_sb[:, SQ * EK // 2:], xqh[:, SQ * EK // 2:])
            for n in range(SQ // 512):
                for m in range(EK):
                    ps = psC.tile([128, 512], F32, tag="psC")
                    for k in range(EK):
                        nc.tensor.matmul(
                            ps[:],
                            wq_sb[:, m * E + k * 128:m * E + (k + 1) * 128],
                            xq_sb[:, n * 4096 + k * 512:n * 4096 + (k + 1) * 512],
                            start=(k == 0), stop=(k == EK - 1))
                    nc.vector.tensor_scalar_add(
                        qt_tiles[m][:, n * 512:(n + 1) * 512],
                        ps[:], bq_t[:, m:m + 1])

        # ======== phase A: V = xvT^T @ Wv + bv ========
        with tc.tile_pool(name="wvp", bufs=1) as wvp, \
             tc.tile_pool(name="xvblk", bufs=2) as xvp, \
             tc.tile_pool(name="psA", bufs=2, space="PSUM") as psA:
            wv_sb = wvp.tile([128, E * EK], BF)
            nc.sync.dma_start(wv_sb[:], wvh)
            for mb in range(SK // 512):
                xv_blk = xvp.tile([128, 512 * EK], BF, tag="xvb",
                                  name=f"xvb{mb}")
                nc.sync.dma_start(
                    xv_blk[:], xvh[:, mb * 512 * EK:(mb + 1) * 512 * EK])
                for n in range(E // 512):
                    pss = [psA.tile([128, 512], F32, tag=f"psA{i}",
                                    name=f"psA_{mb}_{n}_{i}")
                           for i in range(4)]
                    for k in range(EK):
                        for i in range(4):
                            nc.tensor.matmul(
                                pss[i][:],
                                xv_blk[:, k * 512 + i * 128:k * 512 + (i + 1) * 128],
                                wv_sb[:, k * E + n * 512:k * E + (n + 1) * 512],
                                start=(k == 0), stop=(k == EK - 1))
                    for i in range(4):
                        m = mb * 4 + i
                        nc.vector.tensor_add(
                            v_tiles[m][:, n * 512:(n + 1) * 512],
                            pss[i][:],
                            bv_bc[:, n * 512:(n + 1) * 512])

        # ======== phase D: attention, blocked over s_q ========
        with tc.tile_pool(name="partp", bufs=2) as partp, \
             tc.tile_pool(name="outp", bufs=2) as outp, \
             tc.tile_pool(name="sumsp", bufs=4) as sumsp, \
             tc.tile_pool(name="psS", bufs=4, space="PSUM") as psS, \
             tc.tile_pool(name="psO", bufs=3, space="PSUM") as psO, \
             tc.tile_pool(name="psSum", bufs=1, space="PSUM") as psSum:
            for blk in range(NBLK):
                q0 = blk * BQ
                # scoresT = KT^T @ QT_blk; exp (bf16); running sum on DVE
                part = partp.tile([128, BQ], F32, tag="part")
                part_r = partp.tile([128, BQ], F32R, tag="part_r")
                for m in range(MK):
                    ps = psS.tile([128, BQ], F32, tag="psS")
                    for k in range(EK):
                        nc.tensor.matmul(
                            ps[:],
                            kt_tiles[k][:, m * 128:(m + 1) * 128],
                            qt_tiles[k][:, q0:q0 + BQ],
                            start=(k == 0), stop=(k == EK - 1))
                    nc.scalar.activation(exp_tiles[m][:], ps[:], AF.Exp,
                                         scale=INV_SCALE)
                    if m == 1:
                        nc.vector.tensor_add(part[:], exp_tiles[0][:],
                                             exp_tiles[1][:])
                    elif m == MK - 1:
                        nc.vector.tensor_add(part_r[:], part[:],
                                             exp_tiles[m][:])
                    elif m > 1:
                        nc.vector.tensor_add(part[:], part[:], exp_tiles[m][:])

                # out_unnorm[s_q, e] = expT^T @ V ; sums via ones-matmul
                for mi in range(BQ // 128):
                    pssum = psSum.tile([128, 2], F32, tag="pssum")
                    nc.tensor.matmul(
                        pssum[:],
                        part_r[:, mi * 128:(mi + 1) * 128],
                        ones_t, start=True, stop=True)
                    recip = sumsp.tile([128, 1], F32, tag="recip")
                    nc.vector.reciprocal(recip[:], pssum[:, 0:1])

                    ot = outp.tile([128, E], F32, tag="ot")
                    for n in range(E // 512):
                        pso = psO.tile([128, 512], F32, tag="psO")
                        for m in range(MK):
                            nc.tensor.matmul(
                                pso[:],
                                exp_tiles[m][:, mi * 128:(mi + 1) * 128],
                                v_tiles[m][:, n * 512:(n + 1) * 512],
                                start=(m == 0), stop=(m == MK - 1))
                        nc.vector.tensor_scalar_mul(
                            ot[:, n * 512:(n + 1) * 512], pso[:], recip[:])
                        nc.sync.dma_start(
                            out[q0 + mi * 128:q0 + (mi + 1) * 128,
                                n * 512:(n + 1) * 512],
                            ot[:, n * 512:(n + 1) * 512])

    nc.compile()
    return nc


def _get_nc():
    if "nc" not in _cached:
        _cached["nc"] = _build()
    return _cached["nc"]


def _pack_w_mmajor(W):
    # [128, 8(m), 8(k), 128]: [p, m, k, c] = W[k*128+p, m*128+c]
    return np.ascontiguousarray(
        W.reshape(EK, 128, EK, 128).transpose(1, 2, 0, 3).reshape(128, E * EK)
        .astype(NPBF))


def _pack_w_kmajor(W):
    # [128, 8(k), 1024]: [p, k, c] = W[k*128+p, c]
    return np.ascontiguousarray(
        W.reshape(EK, 128, E).transpose(1, 0, 2).reshape(128, E * EK)
        .astype(NPBF))


def _pack_x(x, blk):
    # [128, nb, 8(k), blk]: [p, nb, k, s] = x[nb*blk+s, k*128+p]
    nb = x.shape[0] // blk
    return np.ascontiguousarray(
        x.reshape(nb, blk, EK, 128).transpose(3, 0, 2, 1)
        .reshape(128, x.shape[0] * EK).astype(NPBF))


def kernel(query, key, value, Wq, bq, Wk, bk, Wv, bv, **kw):
    query = np.asarray(query, dtype=np.float32)
    key = np.asarray(key, dtype=np.float32)
    value = np.asarray(value, dtype=np.float32)
    wq_h = _pack_w_mmajor(np.asarray(Wq, dtype=np.float32))
    wk_h = _pack_w_mmajor(np.asarray(Wk, dtype=np.float32))
    wv_h = _pack_w_kmajor(np.asarray(Wv, dtype=np.float32))
    bq = np.asarray(bq, dtype=np.float32)
    bk = np.asarray(bk, dtype=np.float32)
    bv = np.asarray(bv, dtype=np.float32)

    cst_h = np.empty((128, 18), dtype=np.float32)
    cst_h[:, 0:2] = 1.0
    cst_h[:, 2:10] = bq.reshape(EK, 128).T
    cst_h[:, 10:18] = bk.reshape(EK, 128).T
    bv_h = np.ascontiguousarray(
        np.broadcast_to(bv.reshape(1, E), (128, E)).astype(NPBF))

    xk_h = {b: _pack_x(key[b], 512) for b in range(B)}
    xv_h = {b: _pack_x(value[b], 512) for b in range(B)}

    in_maps = []
    for c in range(N_CORES):
        b, h = divmod(c, 2)
        xq_h = _pack_x(query[b, h * SQ:(h + 1) * SQ, :], 512)
        in_maps.append({
            "xqh": xq_h, "xkh": xk_h[b], "xvh": xv_h[b],
            "wqh": wq_h, "wkh": wk_h, "wvh": wv_h,
            "csth": cst_h, "bvh": bv_h,
            "ones_in": np.ones((128, 2), dtype=np.float32),
        })

    nc = _get_nc()
    res = bass_utils.run_bass_kernel_spmd(
        nc, in_maps, core_ids=list(range(N_CORES)), **kw)

    full = np.empty((B, S, E), dtype=np.float32)
    for c in range(N_CORES):
        b, h = divmod(c, 2)
        full[b, h * SQ:(h + 1) * SQ, :] = res.results[c]["out"]
    kernel.last_results = res
    return full


# revision 9
# speedup vs baseline: 1.1932x; 1.1490x over previous
"""Trainium2 Bass kernel for single-head attention model.

Reference computation (B=4, S=2048, E=1024, fp32):
    q = query @ Wq + bq;  k = key @ Wk + bk;  v = value @ Wv + bv
    scores = (q @ k^T) / sqrt(E)
    out = softmax(scores, axis=-1) @ v

Sharding: 8 cores; core c handles batch b = c // 2, query-row half
h = c % 2 (1024 q-rows). K/V projections for the full batch are
computed redundantly on both cores of a pair (no collectives).

v4 strategy:
  - all inputs/weights bf16 (same 1 cyc/row PE rate as f32r, half DMA)
  - host packs every tensor into few large partition-major blocks
    (per-DMA queue overhead is ~0.6us); weights m-major so the first
    output-column group's slices land in one contiguous DMA
  - ALL input tiles live in top-level pools so no phase's DMA waits on
    a previous phase's SBUF reuse (idle PE restarts cost double under
    the p-state ramp)
  - phase order K-proj -> Q-proj -> V-proj -> attention
  - attention: scoresT[s_k, s_q] = KT^T @ QT per 128-row tile, exp via
    ACT (bf16), per-tile running-sum adds interleaved on DVE, softmax
    normalization via ones-matmul partition reduce + reciprocal,
    out_unnorm = expT^T @ V, scaled on eviction, contiguous out DMAs
"""

import sys

sys.path.insert(0, "/opt/trn_rl_repo")

from contextlib import ExitStack

import numpy as np
import ml_dtypes

import concourse.bass as bass
import concourse.mybir as mybir
import concourse.tile as tile
from concourse import bacc, bass_utils

F32R = mybir.dt.float32r
F32 = mybir.dt.float32
BF = mybir.dt.bfloat16
AF = mybir.ActivationFunctionType
NPBF = ml_dtypes.bfloat16

B, S, E = 4, 2048, 1024
N_CORES = 8
SQ = S // 2          # q rows per core
SK = S               # kv rows per core
BQ = 512             # s_q block width in phase D
NBLK = SQ // BQ      # 2 blocks
EK = E // 128        # 8 contraction tiles over e
MK = SK // 128       # 16 s_k tiles
INV_SCALE = 1.0 / float(np.sqrt(E))

_cached = {}


def _build():
    nc = bacc.Bacc("TRN2", target_bir_lowering=False, debug=False,
                   num_devices=N_CORES)

    # xq: [128, 2(n), 8(k), 512]; xk/xv: [128, 4(nb), 8(k), 512]
    xqh = nc.dram_tensor("xqh", [128, SQ * EK], BF, kind="ExternalInput").ap()
    xkh = nc.dram_tensor("xkh", [128, SK * EK], BF, kind="ExternalInput").ap()
    xvh = nc.dram_tensor("xvh", [128, SK * EK], BF, kind="ExternalInput").ap()
    # wq/wk: [128, 8(m), 8(k), 128] m-major; wv: [128, 8(k), 1024]
    wqh = nc.dram_tensor("wqh", [128, E * EK], BF, kind="ExternalInput").ap()
    wkh = nc.dram_tensor("wkh", [128, E * EK], BF, kind="ExternalInput").ap()
    wvh = nc.dram_tensor("wvh", [128, E * EK], BF, kind="ExternalInput").ap()
    # consts: cols [2:10]=bq, [10:18]=bk  (f32)
    csth = nc.dram_tensor("csth", [128, 18], F32, kind="ExternalInput").ap()
    ones_in = nc.dram_tensor("ones_in", [128, 2], F32R, kind="ExternalInput").ap()
    # bv broadcast to all partitions (bf16)
    bvh = nc.dram_tensor("bvh", [128, E], BF, kind="ExternalInput").ap()
    out = nc.dram_tensor("out", [SQ, E], F32, kind="ExternalOutput").ap()

    with tile.TileContext(nc) as tc, ExitStack() as top:
        # ---- all input tiles in long-lived top-level pools ----
        consts = top.enter_context(tc.tile_pool(name="consts", bufs=1))
        ktpool = top.enter_context(tc.tile_pool(name="ktpool", bufs=1))
        qtpool = top.enter_context(tc.tile_pool(name="qtpool", bufs=1))
        vpool = top.enter_context(tc.tile_pool(name="vpool", bufs=1))
        expp = top.enter_context(tc.tile_pool(name="expp", bufs=1))
        wkp = top.enter_context(tc.tile_pool(name="wkp", bufs=1))
        wqp = top.enter_context(tc.tile_pool(name="wqp", bufs=1))
        wvp = top.enter_context(tc.tile_pool(name="wvp", bufs=1))
        xqp = top.enter_context(tc.tile_pool(name="xqp", bufs=1))
        xkp = top.enter_context(tc.tile_pool(name="xkblk", bufs=2))
        xvp = top.enter_context(tc.tile_pool(name="xvblk", bufs=2))

        cst = consts.tile([128, 18], F32)
        nc.sync.dma_start(cst[:], csth)
        ones_t = consts.tile([128, 2], F32R)
        nc.sync.dma_start(ones_t[:], ones_in)
        bq_t = cst[:, 2:10]
        bk_t = cst[:, 10:18]

        # hoist the ACT Exp table load off the critical path
        actwarm = consts.tile([1, 2], F32)
        nc.scalar.activation(actwarm[:], cst[0:1, 0:2], AF.Exp)

        kt_tiles = [ktpool.tile([128, SK], BF, tag=f"kt{m}", name=f"kt{m}")
                    for m in range(EK)]
        qt_tiles = [qtpool.tile([128, SQ], BF, tag=f"qt{m}", name=f"qt{m}")
                    for m in range(EK)]
        v_tiles = [vpool.tile([128, E], BF, tag=f"v{m}", name=f"v{m}")
                   for m in range(MK)]
        exp_tiles = [expp.tile([128, BQ], BF, tag=f"exp{m}", name=f"exp{m}")
                     for m in range(MK)]
        wk_sb = wkp.tile([128, E * EK], BF)
        wq_sb = wqp.tile([128, E * EK], BF)
        wv_sb = wvp.tile([128, E * EK], BF)
        xq_sb = xqp.tile([128, SQ * EK], BF)

        # ======== phase B: KT = Wk^T @ xkT + bk ========
        with tc.tile_pool(name="psB", bufs=8, space="PSUM") as psB:
            # critical-path DMA order: m0 weights, first xk block halves, rest
            nc.sync.dma_start(wk_sb[:, 0:E], wkh[:, 0:E])
            xk_first = xkp.tile([128, 512 * EK], BF, tag="xkb", name="xkb0")
            half = 512 * EK // 2
            nc.sync.dma_start(xk_first[:, 0:half], xkh[:, 0:half])
            nc.sync.dma_start(xk_first[:, half:], xkh[:, half:512 * EK])
            nc.sync.dma_start(wk_sb[:, E:2 * E], wkh[:, E:2 * E])
            nc.sync.dma_start(wk_sb[:, 2 * E:4 * E], wkh[:, 2 * E:4 * E])
            nc.sync.dma_start(wk_sb[:, 4 * E:8 * E], wkh[:, 4 * E:8 * E])
            bv_bc = consts.tile([128, E], BF)
            nc.sync.dma_start(bv_bc[:], bvh)
            for nb in range(SK // 512):
                if nb == 0:
                    xk_blk = xk_first
                else:
                    xk_blk = xkp.tile([128, 512 * EK], BF, tag="xkb",
                                      name=f"xkb{nb}")
                    nc.sync.dma_start(
                        xk_blk[:], xkh[:, nb * 512 * EK:(nb + 1) * 512 * EK])
                for m in range(EK):
                    ps = psB.tile([128, 512], F32, tag="psB")
                    for k in range(EK):
                        nc.tensor.matmul(
                            ps[:],
                            wk_sb[:, m * E + k * 128:m * E + (k + 1) * 128],
                            xk_blk[:, k * 512:(k + 1) * 512],
                            start=(k == 0), stop=(k == EK - 1))
                    nc.vector.tensor_scalar_add(
                        kt_tiles[m][:, nb * 512:(nb + 1) * 512],
                        ps[:], bk_t[:, m:m + 1])

        # ======== phase C: QT = Wq^T @ xqT + bq ========
        with tc.tile_pool(name="psC", bufs=8, space="PSUM") as psC:
            nc.sync.dma_start(wq_sb[:, 0:2 * E], wqh[:, 0:2 * E])
            nc.sync.dma_start(xq_sb[:, 0:SQ * EK // 2], xqh[:, 0:SQ * EK // 2])
            nc.sync.dma_start(wq_sb[:, 2 * E:8 * E], wqh[:, 2 * E:8 * E])
            nc.sync.dma_start(xq_sb[:, SQ * EK // 2:], xqh[:, SQ * EK // 2:])
            for n in range(SQ // 512):
                for m in range(EK):
                    ps = psC.tile([128, 512], F32, tag="psC")
                    for k in range(EK):
                        nc.tensor.matmul(
                            ps[:],
                            wq_sb[:, m * E + k * 128:m * E + (k + 1) * 128],
                            xq_sb[:, n * 4096 + k * 512:n * 4096 + (k + 1) * 512],
                            start=(k == 0), stop=(k == EK - 1))
                    nc.vector.tensor_scalar_add(
                        qt_tiles[m][:, n * 512:(n + 1) * 512],
                        ps[:], bq_t[:, m:m + 1])

        # ======== phase A: V = xvT^T @ Wv + bv ========
        with tc.tile_pool(name="psA", bufs=2, space="PSUM") as psA:
            nc.sync.dma_start(wv_sb[:], wvh)
            for mb in range(SK // 512):
                xv_blk = xvp.tile([128, 512 * EK], BF, tag="xvb",
                                  name=f"xvb{mb}")
                nc.sync.dma_start(
                    xv_blk[:], xvh[:, mb * 512 * EK:(mb + 1) * 512 * EK])
                for n in range(E // 512):
                    pss = [psA.tile([128, 512], F32, tag=f"psA{i}",
                                    name=f"psA_{mb}_{n}_{i}")
                           for i in range(4)]
                    for k in range(EK):
                        for i in range(4):
                            nc.tensor.matmul(
                                pss[i][:],
                                xv_blk[:, k * 512 + i * 128:k * 512 + (i + 1) * 128],
                                wv_sb[:, k * E + n * 512:k * E + (n + 1) * 512],
                                start=(k == 0), stop=(k == EK - 1))
                    for i in range(4):
                        m = mb * 4 + i
                        nc.vector.tensor_add(
                            v_tiles[m][:, n * 512:(n + 1) * 512],
                            pss[i][:],
                            bv_bc[:, n * 512:(n + 1) * 512])

        # ======== phase D: attention, blocked over s_q ========
        with tc.tile_pool(name="partp", bufs=1) as partp, \
             tc.tile_pool(name="outp", bufs=2) as outp, \
             tc.tile_pool(name="sumsp", bufs=4) as sumsp, \
             tc.tile_pool(name="psS", bufs=4, space="PSUM") as psS, \
             tc.tile_pool(name="psO", bufs=3, space="PSUM") as psO, \
             tc.tile_pool(name="psSum", bufs=1, space="PSUM") as psSum:
            for blk in range(NBLK):
                q0 = blk * BQ
                # scoresT = KT^T @ QT_blk; exp (bf16); running sum on DVE
                part = partp.tile([128, BQ], F32, tag="part")
                part_r = partp.tile([128, BQ], F32R, tag="part_r")
                for m in range(MK):
                    ps = psS.tile([128, BQ], F32, tag="psS")
                    for k in range(EK):
                        nc.tensor.matmul(
                            ps[:],
                            kt_tiles[k][:, m * 128:(m + 1) * 128],
                            qt_tiles[k][:, q0:q0 + BQ],
                            start=(k == 0), stop=(k == EK - 1))
                    nc.scalar.activation(exp_tiles[m][:], ps[:], AF.Exp,
                                         scale=INV_SCALE)
                    if m == 1:
                        nc.vector.tensor_add(part[:], exp_tiles[0][:],
                                             exp_tiles[1][:])
                    elif m == MK - 1:
                        nc.vector.tensor_add(part_r[:], part[:],
                                             exp_tiles[m][:])
                    elif m > 1:
                        nc.vector.tensor_add(part[:], part[:], exp_tiles[m][:])

                # out_unnorm[s_q, e] = expT^T @ V ; sums via ones-matmul
                for mi in range(BQ // 128):
                    pssum = psSum.tile([128, 2], F32, tag="pssum")
                    nc.tensor.matmul(
                        pssum[:],
                        part_r[:, mi * 128:(mi + 1) * 128],
                        ones_t[:], start=True, stop=True)
                    recip = sumsp.tile([128, 1], F32, tag="recip")
                    nc.vector.reciprocal(recip[:], pssum[:, 0:1])

                    ot = outp.tile([128, E], F32, tag="ot")
                    for n in range(E // 512):
                        pso = psO.tile([128, 512], F32, tag="psO")
                        for m in range(MK):
                            nc.tensor.matmul(
                                pso[:],
                                exp_tiles[m][:, mi * 128:(mi + 1) * 128],
                                v_tiles[m][:, n * 512:(n + 1) * 512],
                                start=(m == 0), stop=(m == MK - 1))
                        nc.vector.tensor_scalar_mul(
                            ot[:, n * 512:(n + 1) * 512], pso[:], recip[:])
                    nc.sync.dma_start(
                        out[q0 + mi * 128:q0 + (mi + 1) * 128, :], ot[:])

    nc.compile()
    return nc


def _get_nc():
    if "nc" not in _cached:
        _cached["nc"] = _build()
    return _cached["nc"]


def _pack_w_mmajor(W):
    # [128, 8(m), 8(k), 128]: [p, m, k, c] = W[k*128+p, m*128+c]
    return np.ascontiguousarray(
        W.reshape(EK, 128, EK, 128).transpose(1, 2, 0, 3).reshape(128, E * EK)
        .astype(NPBF))


def _pack_w_kmajor(W):
    # [128, 8(k), 1024]: [p, k, c] = W[k*128+p, c]
    return np.ascontiguousarray(
        W.reshape(EK, 128, E).transpose(1, 0, 2).reshape(128, E * EK)
        .astype(NPBF))


def _pack_x(x, blk):
    # [128, nb, 8(k), blk]: [p, nb, k, s] = x[nb*blk+s, k*128+p]
    nb = x.shape[0] // blk
    return np.ascontiguousarray(
        x.reshape(nb, blk, EK, 128).transpose(3, 0, 2, 1)
        .reshape(128, x.shape[0] * EK).astype(NPBF))


def kernel(query, key, value, Wq, bq, Wk, bk, Wv, bv, **kw):
    query = np.asarray(query, dtype=np.float32)
    key = np.asarray(key, dtype=np.float32)
    value = np.asarray(value, dtype=np.float32)
    wq_h = _pack_w_mmajor(np.asarray(Wq, dtype=np.float32))
    wk_h = _pack_w_mmajor(np.asarray(Wk, dtype=np.float32))
    wv_h = _pack_w_kmajor(np.asarray(Wv, dtype=np.float32))
    bq = np.asarray(bq, dtype=np.float32)
    bk = np.asarray(bk, dtype=np.float32)
    bv = np.asarray(bv, dtype=np.float32)

    cst_h = np.empty((128, 18), dtype=np.float32)
    cst_h[:, 0:2] = 1.0
    cst_h[:, 2:10] = bq.reshape(EK, 128).T
    cst_h[:, 10:18] = bk.reshape(EK, 128).T
    bv_h = np.ascontiguousarray(
        np.broadcast_to(bv.reshape(1, E), (128, E)).astype(NPBF))

    xk_h = {b: _pack_x(key[b], 512) for b in range(B)}
    xv_h = {b: _pack_x(value[b], 512) for b in range(B)}

    in_maps = []
    for c in range(N_CORES):
        b, h = divmod(c, 2)
        xq_h = _pack_x(query[b, h * SQ:(h + 1) * SQ, :], 512)
        in_maps.append({
            "xqh": xq_h, "xkh": xk_h[b], "xvh": xv_h[b],
            "wqh": wq_h, "wkh": wk_h, "wvh": wv_h,
            "csth": cst_h, "bvh": bv_h,
            "ones_in": np.ones((128, 2), dtype=np.float32),
        })

    nc = _get_nc()
    res = bass_utils.run_bass_kernel_spmd(
        nc, in_maps, core_ids=list(range(N_CORES)), **kw)

    full = np.empty((B, S, E), dtype=np.float32)
    for c in range(N_CORES):
        b, h = divmod(c, 2)
        full[b, h * SQ:(h + 1) * SQ, :] = res.results[c]["out"]
    kernel.last_results = res
    return full


# revision 10
# speedup vs baseline: 1.4201x; 1.1902x over previous
"""Trainium2 Bass kernel for single-head attention model.

Reference computation (B=4, S=2048, E=1024, fp32):
    q = query @ Wq + bq;  k = key @ Wk + bk;  v = value @ Wv + bv
    scores = (q @ k^T) / sqrt(E)
    out = softmax(scores, axis=-1) @ v

Sharding: 8 cores; core c handles batch b = c // 2, query-row half
h = c % 2 (1024 q-rows). K/V projections for the full batch are
computed redundantly on both cores of a pair (no collectives).

v5 strategy:
  - inputs/weights bf16; host packs everything partition-major so each
    phase needs a handful of large DMAs; weights m-major
  - all input tiles in top-level pools (no SBUF-reuse anti-deps) and
    ONE shared PSUM rotation for every 8-matmul accumulation group so
    no phase boundary ever stalls the PE
  - phase order K-proj -> V-proj -> Q-proj -> attention
  - first K-proj block processed in two 256-wide halves so the first
    matmul starts as early as possible
  - QT/KT written as fp8e4 in k-tile pairs; the scores matmul runs
    perf_mode=DoubleRow (K=256 per instruction, ~2x PE rate)
  - attention: exp via ACT (bf16), per-tile running-sum adds on DVE
    interleaved with the score groups, normalization via ones-matmul
    partition reduce + reciprocal applied on psum eviction
  - out written [128, 8, E] partition-major, per-512-slice DMAs
"""

import sys

sys.path.insert(0, "/opt/trn_rl_repo")

from contextlib import ExitStack

import numpy as np
import ml_dtypes

import concourse.bass as bass
import concourse.mybir as mybir
import concourse.tile as tile
from concourse import bacc, bass_utils

F32R = mybir.dt.float32r
F32 = mybir.dt.float32
BF = mybir.dt.bfloat16
FP8 = mybir.dt.float8e4
AF = mybir.ActivationFunctionType
DR = mybir.MatmulPerfMode.DoubleRow
NPBF = ml_dtypes.bfloat16

B, S, E = 4, 2048, 1024
N_CORES = 8
SQ = S // 2          # q rows per core
SK = S               # kv rows per core
BQ = 512             # s_q block width in phase D
NBLK = SQ // BQ      # 2 blocks
EK = E // 128        # 8 contraction tiles over e
MK = SK // 128       # 16 s_k tiles
INV_SCALE = 1.0 / float(np.sqrt(E))

USE_FP8_SCORES = True

_cached = {}


def _build():
    nc = bacc.Bacc("TRN2", target_bir_lowering=False, debug=False,
                   num_devices=N_CORES)

    kq_dt = FP8 if USE_FP8_SCORES else BF

    # x layouts: [128, nb, 8(k), 512]
    xqh = nc.dram_tensor("xqh", [128, SQ // 512, EK, 512], BF,
                         kind="ExternalInput").ap()
    xkh = nc.dram_tensor("xkh", [128, SK // 512, EK, 512], BF,
                         kind="ExternalInput").ap()
    xvh = nc.dram_tensor("xvh", [128, SK // 512, EK, 512], BF,
                         kind="ExternalInput").ap()
    # wq/wk: [128, 8(m), 8(k), 128] m-major; wv: [128, 8(k), 1024]
    wqh = nc.dram_tensor("wqh", [128, E * EK], BF, kind="ExternalInput").ap()
    wkh = nc.dram_tensor("wkh", [128, E * EK], BF, kind="ExternalInput").ap()
    wvh = nc.dram_tensor("wvh", [128, E * EK], BF, kind="ExternalInput").ap()
    # consts: cols [2:10]=bq, [10:18]=bk  (f32)
    csth = nc.dram_tensor("csth", [128, 18], F32, kind="ExternalInput").ap()
    ones_in = nc.dram_tensor("ones_in", [128, 2], F32R, kind="ExternalInput").ap()
    # bv broadcast to all partitions (bf16)
    bvh = nc.dram_tensor("bvh", [128, E], BF, kind="ExternalInput").ap()
    # out row-tiles g=0..7 partition-major: [p, g, e] = out[g*128+p, e]
    out = nc.dram_tensor("out", [128, SQ // 128, E], F32,
                         kind="ExternalOutput").ap()

    with tile.TileContext(nc) as tc, ExitStack() as top:
        # ---- pools (all top-level) ----
        consts = top.enter_context(tc.tile_pool(name="consts", bufs=1))
        ktpool = top.enter_context(tc.tile_pool(name="ktpool", bufs=1))
        qtpool = top.enter_context(tc.tile_pool(name="qtpool", bufs=1))
        vpool = top.enter_context(tc.tile_pool(name="vpool", bufs=1))
        expp = top.enter_context(tc.tile_pool(name="expp", bufs=1))
        wkp = top.enter_context(tc.tile_pool(name="wkp", bufs=1))
        wqp = top.enter_context(tc.tile_pool(name="wqp", bufs=1))
        wvp = top.enter_context(tc.tile_pool(name="wvp", bufs=1))
        xqp = top.enter_context(tc.tile_pool(name="xqp", bufs=1))
        xkp = top.enter_context(tc.tile_pool(name="xkblk", bufs=2))
        xvp = top.enter_context(tc.tile_pool(name="xvblk", bufs=2))
        partp = top.enter_context(tc.tile_pool(name="partp", bufs=1))
        outp = top.enter_context(tc.tile_pool(name="outp", bufs=2))
        sumsp = top.enter_context(tc.tile_pool(name="sumsp", bufs=4))
        psMain = top.enter_context(tc.tile_pool(name="psMain", bufs=4,
                                                space="PSUM"))
        psO = top.enter_context(tc.tile_pool(name="psO", bufs=3, space="PSUM"))
        psSum = top.enter_context(tc.tile_pool(name="psSum", bufs=1,
                                               space="PSUM"))

        cst = consts.tile([128, 18], F32)
        nc.sync.dma_start(cst[:], csth)
        ones_t = consts.tile([128, 2], F32R)
        nc.sync.dma_start(ones_t[:], ones_in)
        bq_t = cst[:, 2:10]
        bk_t = cst[:, 10:18]

        # hoist the ACT Exp table load off the critical path
        actwarm = consts.tile([1, 2], F32)
        nc.scalar.activation(actwarm[:], cst[0:1, 0:2], AF.Exp)

        # KT/QT as k-tile PAIRS for DoubleRow: [128, 2, cols]
        kt_pair = [ktpool.tile([128, 2, SK], kq_dt, tag=f"kt{j}", name=f"kt{j}")
                   for j in range(EK // 2)]
        qt_pair = [qtpool.tile([128, 2, SQ], kq_dt, tag=f"qt{j}", name=f"qt{j}")
                   for j in range(EK // 2)]
        v_tiles = [vpool.tile([128, E], BF, tag=f"v{m}", name=f"v{m}")
                   for m in range(MK)]
        exp_tiles = [expp.tile([128, BQ], BF, tag=f"exp{m}", name=f"exp{m}")
                     for m in range(MK)]
        wk_sb = wkp.tile([128, E * EK], BF)
        wq_sb = wqp.tile([128, E * EK], BF)
        wv_sb = wvp.tile([128, E * EK], BF)
        xq_sb = xqp.tile([128, SQ // 512, EK, 512], BF)

        def proj_group(ps_slice, w_sb, x_ap, m, width):
            for k in range(EK):
                nc.tensor.matmul(
                    ps_slice,
                    w_sb[:, m * E + k * 128:m * E + (k + 1) * 128],
                    x_ap(k)[:, 0:width] if callable(x_ap) else x_ap,
                    start=(k == 0), stop=(k == EK - 1))

        # ======== phase B: KT = Wk^T @ xkT + bk ========
        # critical-path DMA order: m0 weights, first half-block, rest
        nc.sync.dma_start(wk_sb[:, 0:E], wkh[:, 0:E])
        xk_first = xkp.tile([128, EK, 512], BF, tag="xkb", name="xkb0")
        nc.sync.dma_start(xk_first[:, :, 0:256], xkh[:, 0, :, 0:256])
        nc.sync.dma_start(xk_first[:, :, 256:512], xkh[:, 0, :, 256:512])
        nc.sync.dma_start(wk_sb[:, E:2 * E], wkh[:, E:2 * E])
        nc.sync.dma_start(wk_sb[:, 2 * E:4 * E], wkh[:, 2 * E:4 * E])
        nc.sync.dma_start(wk_sb[:, 4 * E:8 * E], wkh[:, 4 * E:8 * E])
        bv_bc = consts.tile([128, E], BF)
        nc.sync.dma_start(bv_bc[:], bvh)
        for nb in range(SK // 512):
            if nb == 0:
                xk_blk = xk_first
                # two 256-wide halves so the first matmul starts early
                spans = [(0, 256), (256, 512)]
            else:
                xk_blk = xkp.tile([128, EK, 512], BF, tag="xkb",
                                  name=f"xkb{nb}")
                nc.sync.dma_start(xk_blk[:], xkh[:, nb])
                spans = [(0, 512)]
            for s0, s1 in spans:
                for m in range(EK):
                    w = s1 - s0
                    ps = psMain.tile([128, 512], F32, tag="ps")
                    for k in range(EK):
                        nc.tensor.matmul(
                            ps[:, 0:w],
                            wk_sb[:, m * E + k * 128:m * E + (k + 1) * 128],
                            xk_blk[:, k, s0:s1],
                            start=(k == 0), stop=(k == EK - 1))
                    j, kk = divmod(m, 2)
                    nc.vector.tensor_scalar_add(
                        kt_pair[j][:, kk:kk + 1, nb * 512 + s0:nb * 512 + s1],
                        ps[:, 0:w], bk_t[:, m:m + 1])

        # ======== phase A: V = xvT^T @ Wv + bv ========
        nc.sync.dma_start(wv_sb[:], wvh)
        for mb in range(SK // 512):
            xv_blk = xvp.tile([128, EK, 512], BF, tag="xvb", name=f"xvb{mb}")
            nc.sync.dma_start(xv_blk[:], xvh[:, mb])
            for n in range(E // 512):
                for i in range(4):
                    ps = psMain.tile([128, 512], F32, tag="ps")
                    for k in range(EK):
                        nc.tensor.matmul(
                            ps[:],
                            xv_blk[:, k, i * 128:(i + 1) * 128],
                            wv_sb[:, k * E + n * 512:k * E + (n + 1) * 512],
                            start=(k == 0), stop=(k == EK - 1))
                    nc.vector.tensor_add(
                        v_tiles[mb * 4 + i][:, n * 512:(n + 1) * 512],
                        ps[:], bv_bc[:, n * 512:(n + 1) * 512])

        # ======== phase C: QT = Wq^T @ xqT + bq ========
        nc.sync.dma_start(wq_sb[:, 0:2 * E], wqh[:, 0:2 * E])
        nc.sync.dma_start(xq_sb[:, 0], xqh[:, 0])
        nc.sync.dma_start(wq_sb[:, 2 * E:8 * E], wqh[:, 2 * E:8 * E])
        nc.sync.dma_start(xq_sb[:, 1], xqh[:, 1])
        for n in range(SQ // 512):
            for m in range(EK):
                ps = psMain.tile([128, 512], F32, tag="ps")
                for k in range(EK):
                    nc.tensor.matmul(
                        ps[:],
                        wq_sb[:, m * E + k * 128:m * E + (k + 1) * 128],
                        xq_sb[:, n, k],
                        start=(k == 0), stop=(k == EK - 1))
                j, kk = divmod(m, 2)
                nc.vector.tensor_scalar_add(
                    qt_pair[j][:, kk:kk + 1, n * 512:(n + 1) * 512],
                    ps[:], bq_t[:, m:m + 1])

        # ======== phase D: attention, blocked over s_q ========
        for blk in range(NBLK):
            q0 = blk * BQ
            # scoresT = KT^T @ QT_blk; exp (bf16); running sum on DVE
            part = partp.tile([128, BQ], F32, tag="part")
            part_r = partp.tile([128, BQ], F32R, tag="part_r")
            for m in range(MK):
                ps = psMain.tile([128, 512], F32, tag="ps")
                if USE_FP8_SCORES:
                    for j in range(EK // 2):
                        nc.tensor.matmul(
                            ps[:],
                            kt_pair[j][:, 0:2, m * 128:(m + 1) * 128],
                            qt_pair[j][:, 0:2, q0:q0 + BQ],
                            start=(j == 0), stop=(j == EK // 2 - 1),
                            perf_mode=DR)
                else:
                    for j in range(EK // 2):
                        for kk in range(2):
                            nc.tensor.matmul(
                                ps[:],
                                kt_pair[j][:, kk, m * 128:(m + 1) * 128],
                                qt_pair[j][:, kk, q0:q0 + BQ],
                                start=(j == 0 and kk == 0),
                                stop=(j == EK // 2 - 1 and kk == 1))
                nc.scalar.activation(exp_tiles[m][:], ps[:], AF.Exp,
                                     scale=INV_SCALE)
                if m == 1:
                    nc.vector.tensor_add(part[:], exp_tiles[0][:],
                                         exp_tiles[1][:])
                elif m == MK - 1:
                    nc.vector.tensor_add(part_r[:], part[:], exp_tiles[m][:])
                elif m > 1:
                    nc.vector.tensor_add(part[:], part[:], exp_tiles[m][:])

            # out_unnorm[s_q, e] = expT^T @ V ; sums via ones-matmul
            for mi in range(BQ // 128):
                g = blk * (BQ // 128) + mi
                pssum = psSum.tile([128, 2], F32, tag="pssum")
                nc.tensor.matmul(
                    pssum[:],
                    part_r[:, mi * 128:(mi + 1) * 128],
                    ones_t[:], start=True, stop=True)
                recip = sumsp.tile([128, 1], F32, tag="recip")
                nc.vector.reciprocal(recip[:], pssum[:, 0:1])

                ot = outp.tile([128, E], F32, tag="ot")
                for n in range(E // 512):
                    pso = psO.tile([128, 512], F32, tag="psO")
                    for m in range(MK):
                        nc.tensor.matmul(
                            pso[:],
                            exp_tiles[m][:, mi * 128:(mi + 1) * 128],
                            v_tiles[m][:, n * 512:(n + 1) * 512],
                            start=(m == 0), stop=(m == MK - 1))
                    nc.vector.tensor_scalar_mul(
                        ot[:, n * 512:(n + 1) * 512], pso[:], recip[:])
                    nc.sync.dma_start(
                        out[:, g, n * 512:(n + 1) * 512],
                        ot[:, n * 512:(n + 1) * 512])

    nc.compile()
    return nc


def _get_nc():
    if "nc" not in _cached:
        _cached["nc"] = _build()
    return _cached["nc"]


def _pack_w_mmajor(W):
    # [128, 8(m), 8(k), 128]: [p, m, k, c] = W[k*128+p, m*128+c]
    return np.ascontiguousarray(
        W.reshape(EK, 128, EK, 128).transpose(1, 2, 0, 3).reshape(128, E * EK)
        .astype(NPBF))


def _pack_w_kmajor(W):
    # [128, 8(k), 1024]: [p, k, c] = W[k*128+p, c]
    return np.ascontiguousarray(
        W.reshape(EK, 128, E).transpose(1, 0, 2).reshape(128, E * EK)
        .astype(NPBF))


def _pack_x(x, blk):
    # [128, nb, 8(k), blk]: [p, nb, k, s] = x[nb*blk+s, k*128+p]
    nb = x.shape[0] // blk
    return np.ascontiguousarray(
        x.reshape(nb, blk, EK, 128).transpose(3, 0, 2, 1).astype(NPBF))


def kernel(query, key, value, Wq, bq, Wk, bk, Wv, bv, **kw):
    query = np.asarray(query, dtype=np.float32)
    key = np.asarray(key, dtype=np.float32)
    value = np.asarray(value, dtype=np.float32)
    wq_h = _pack_w_mmajor(np.asarray(Wq, dtype=np.float32))
    wk_h = _pack_w_mmajor(np.asarray(Wk, dtype=np.float32))
    wv_h = _pack_w_kmajor(np.asarray(Wv, dtype=np.float32))
    bq = np.asarray(bq, dtype=np.float32)
    bk = np.asarray(bk, dtype=np.float32)
    bv = np.asarray(bv, dtype=np.float32)

    cst_h = np.empty((128, 18), dtype=np.float32)
    cst_h[:, 0:2] = 1.0
    cst_h[:, 2:10] = bq.reshape(EK, 128).T
    cst_h[:, 10:18] = bk.reshape(EK, 128).T
    bv_h = np.ascontiguousarray(
        np.broadcast_to(bv.reshape(1, E), (128, E)).astype(NPBF))

    xk_h = {b: _pack_x(key[b], 512) for b in range(B)}
    xv_h = {b: _pack_x(value[b], 512) for b in range(B)}

    in_maps = []
    for c in range(N_CORES):
        b, h = divmod(c, 2)
        xq_h = _pack_x(query[b, h * SQ:(h + 1) * SQ, :], 512)
        in_maps.append({
            "xqh": xq_h, "xkh": xk_h[b], "xvh": xv_h[b],
            "wqh": wq_h, "wkh": wk_h, "wvh": wv_h,
            "csth": cst_h, "bvh": bv_h,
            "ones_in": np.ones((128, 2), dtype=np.float32),
        })

    nc = _get_nc()
    res = bass_utils.run_bass_kernel_spmd(
        nc, in_maps, core_ids=list(range(N_CORES)), **kw)

    full = np.empty((B, S, E), dtype=np.float32)
    for c in range(N_CORES):
        b, h = divmod(c, 2)
        # out [128, 8, E] -> [SQ, E]
        o = res.results[c]["out"]
        full[b, h * SQ:(h + 1) * SQ, :] = o.transpose(1, 0, 2).reshape(SQ, E)
    kernel.last_results = res
    return full


# revision 13
# speedup vs baseline: 1.4277x; 1.0053x over previous
"""Trainium2 Bass kernel for single-head attention model.

Reference computation (B=4, S=2048, E=1024, fp32):
    q = query @ Wq + bq;  k = key @ Wk + bk;  v = value @ Wv + bv
    scores = (q @ k^T) / sqrt(E)
    out = softmax(scores, axis=-1) @ v

Sharding: 8 cores; core c handles batch b = c // 2, query-row half
h = c % 2 (1024 q-rows). K/V projections for the full batch are
computed redundantly on both cores of a pair (no collectives).

v5 strategy:
  - inputs/weights bf16; host packs everything partition-major so each
    phase needs a handful of large DMAs; weights m-major
  - all input tiles in top-level pools (no SBUF-reuse anti-deps) and
    ONE shared PSUM rotation for every 8-matmul accumulation group so
    no phase boundary ever stalls the PE
  - phase order K-proj -> V-proj -> Q-proj -> attention
  - first K-proj block processed in two 256-wide halves so the first
    matmul starts as early as possible
  - QT/KT written as fp8e4 in k-tile pairs; the scores matmul runs
    perf_mode=DoubleRow (K=256 per instruction, ~2x PE rate)
  - attention: exp via ACT (bf16), per-tile running-sum adds on DVE
    interleaved with the score groups, normalization via ones-matmul
    partition reduce + reciprocal applied on psum eviction
  - out written [128, 8, E] partition-major, per-512-slice DMAs
"""

import sys

sys.path.insert(0, "/opt/trn_rl_repo")

from contextlib import ExitStack

import numpy as np
import ml_dtypes

import concourse.bass as bass
import concourse.mybir as mybir
import concourse.tile as tile
from concourse import bacc, bass_utils

F32R = mybir.dt.float32r
F32 = mybir.dt.float32
BF = mybir.dt.bfloat16
FP8 = mybir.dt.float8e4
AF = mybir.ActivationFunctionType
DR = mybir.MatmulPerfMode.DoubleRow
NPBF = ml_dtypes.bfloat16

B, S, E = 4, 2048, 1024
N_CORES = 8
SQ = S // 2          # q rows per core
SK = S               # kv rows per core
BQ = 512             # s_q block width in phase D
NBLK = SQ // BQ      # 2 blocks
EK = E // 128        # 8 contraction tiles over e
MK = SK // 128       # 16 s_k tiles
INV_SCALE = 1.0 / float(np.sqrt(E))

USE_FP8_SCORES = True

_cached = {}


def _build():
    nc = bacc.Bacc("TRN2", target_bir_lowering=False, debug=False,
                   num_devices=N_CORES)

    kq_dt = FP8 if USE_FP8_SCORES else BF

    # x layouts: [128, nb, 8(k), 512]
    xqh = nc.dram_tensor("xqh", [128, SQ // 512, EK, 512], BF,
                         kind="ExternalInput").ap()
    xkh = nc.dram_tensor("xkh", [128, SK // 512, EK, 512], BF,
                         kind="ExternalInput").ap()
    xvh = nc.dram_tensor("xvh", [128, SK // 512, EK, 512], BF,
                         kind="ExternalInput").ap()
    # wq/wk: [128, 8(m), 8(k), 128] m-major; wv: [128, 8(k), 1024]
    wqh = nc.dram_tensor("wqh", [128, E * EK], BF, kind="ExternalInput").ap()
    wkh = nc.dram_tensor("wkh", [128, E * EK], BF, kind="ExternalInput").ap()
    wvh = nc.dram_tensor("wvh", [128, E * EK], BF, kind="ExternalInput").ap()
    # consts: cols [2:10]=bq, [10:18]=bk  (f32)
    csth = nc.dram_tensor("csth", [128, 18], F32, kind="ExternalInput").ap()
    ones_in = nc.dram_tensor("ones_in", [128, 2], F32R, kind="ExternalInput").ap()
    # bv broadcast to all partitions (bf16)
    bvh = nc.dram_tensor("bvh", [128, E], BF, kind="ExternalInput").ap()
    # out row-tiles g=0..7 partition-major: [p, g, e] = out[g*128+p, e]
    out = nc.dram_tensor("out", [128, SQ // 128, E], F32,
                         kind="ExternalOutput").ap()

    with tile.TileContext(nc) as tc, ExitStack() as top:
        # ---- pools (all top-level) ----
        consts = top.enter_context(tc.tile_pool(name="consts", bufs=1))
        ktpool = top.enter_context(tc.tile_pool(name="ktpool", bufs=1))
        qtpool = top.enter_context(tc.tile_pool(name="qtpool", bufs=1))
        vpool = top.enter_context(tc.tile_pool(name="vpool", bufs=1))
        expp = top.enter_context(tc.tile_pool(name="expp", bufs=1))
        wkp = top.enter_context(tc.tile_pool(name="wkp", bufs=1))
        wqp = top.enter_context(tc.tile_pool(name="wqp", bufs=1))
        wvp = top.enter_context(tc.tile_pool(name="wvp", bufs=1))
        xqp = top.enter_context(tc.tile_pool(name="xqp", bufs=1))
        xkp = top.enter_context(tc.tile_pool(name="xkblk", bufs=2))
        xvp = top.enter_context(tc.tile_pool(name="xvblk", bufs=2))
        partp = top.enter_context(tc.tile_pool(name="partp", bufs=1))
        outp = top.enter_context(tc.tile_pool(name="outp", bufs=2))
        sumsp = top.enter_context(tc.tile_pool(name="sumsp", bufs=4))
        psMain = top.enter_context(tc.tile_pool(name="psMain", bufs=4,
                                                space="PSUM"))
        psO = top.enter_context(tc.tile_pool(name="psO", bufs=3, space="PSUM"))
        psSum = top.enter_context(tc.tile_pool(name="psSum", bufs=1,
                                               space="PSUM"))

        cst = consts.tile([128, 18], F32)
        ones_t = consts.tile([128, 2], F32R)
        bq_t = cst[:, 2:10]
        bk_t = cst[:, 10:18]

        # KT/QT as k-tile PAIRS for DoubleRow: [128, 2, cols]
        kt_pair = [ktpool.tile([128, 2, SK], kq_dt, tag=f"kt{j}", name=f"kt{j}")
                   for j in range(EK // 2)]
        qt_pair = [qtpool.tile([128, 2, SQ], kq_dt, tag=f"qt{j}", name=f"qt{j}")
                   for j in range(EK // 2)]
        v_tiles = [vpool.tile([128, E], BF, tag=f"v{m}", name=f"v{m}")
                   for m in range(MK)]
        exp_tiles = [expp.tile([128, BQ], BF, tag=f"exp{m}", name=f"exp{m}")
                     for m in range(MK)]
        wk_sb = wkp.tile([128, E * EK], BF)
        wq_sb = wqp.tile([128, E * EK], BF)
        wv_sb = wvp.tile([128, E * EK], BF)
        xq_sb = xqp.tile([128, SQ // 512, EK, 512], BF)

        def proj_group(ps_slice, w_sb, x_ap, m, width):
            for k in range(EK):
                nc.tensor.matmul(
                    ps_slice,
                    w_sb[:, m * E + k * 128:m * E + (k + 1) * 128],
                    x_ap(k)[:, 0:width] if callable(x_ap) else x_ap,
                    start=(k == 0), stop=(k == EK - 1))

        # ======== phase B: KT = Wk^T @ xkT + bk ========
        # critical-path DMA order: m0 weights, first half-block, rest
        nc.sync.dma_start(wk_sb[:, 0:E], wkh[:, 0:E])
        xk_first = xkp.tile([128, EK, 512], BF, tag="xkb", name="xkb0")
        nc.sync.dma_start(xk_first[:, :, 0:256], xkh[:, 0, :, 0:256])
        nc.sync.dma_start(xk_first[:, :, 256:512], xkh[:, 0, :, 256:512])
        nc.sync.dma_start(wk_sb[:, E:2 * E], wkh[:, E:2 * E])
        nc.sync.dma_start(cst[:], csth)
        nc.sync.dma_start(ones_t[:], ones_in)
        nc.sync.dma_start(wk_sb[:, 2 * E:4 * E], wkh[:, 2 * E:4 * E])
        nc.sync.dma_start(wk_sb[:, 4 * E:8 * E], wkh[:, 4 * E:8 * E])
        bv_bc = consts.tile([128, E], BF)
        nc.sync.dma_start(bv_bc[:], bvh)

        # hoist the ACT Exp table load off the critical path
        actwarm = consts.tile([1, 2], F32)
        nc.scalar.activation(actwarm[:], cst[0:1, 0:2], AF.Exp)
        for nb in range(SK // 512):
            if nb == 0:
                xk_blk = xk_first
                # two 256-wide halves so the first matmul starts early
                spans = [(0, 256), (256, 512)]
            else:
                xk_blk = xkp.tile([128, EK, 512], BF, tag="xkb",
                                  name=f"xkb{nb}")
                nc.sync.dma_start(xk_blk[:], xkh[:, nb])
                spans = [(0, 512)]
            for s0, s1 in spans:
                for m in range(EK):
                    w = s1 - s0
                    ps = psMain.tile([128, 512], F32, tag="ps")
                    for k in range(EK):
                        nc.tensor.matmul(
                            ps[:, 0:w],
                            wk_sb[:, m * E + k * 128:m * E + (k + 1) * 128],
                            xk_blk[:, k, s0:s1],
                            start=(k == 0), stop=(k == EK - 1))
                    j, kk = divmod(m, 2)
                    nc.vector.tensor_scalar_add(
                        kt_pair[j][:, kk:kk + 1, nb * 512 + s0:nb * 512 + s1],
                        ps[:, 0:w], bk_t[:, m:m + 1])

        # ======== phase A: V = xvT^T @ Wv + bv ========
        nc.sync.dma_start(wv_sb[:], wvh)
        for mb in range(SK // 512):
            xv_blk = xvp.tile([128, EK, 512], BF, tag="xvb", name=f"xvb{mb}")
            nc.sync.dma_start(xv_blk[:], xvh[:, mb])
            for n in range(E // 512):
                for i in range(4):
                    ps = psMain.tile([128, 512], F32, tag="ps")
                    for k in range(EK):
                        nc.tensor.matmul(
                            ps[:],
                            xv_blk[:, k, i * 128:(i + 1) * 128],
                            wv_sb[:, k * E + n * 512:k * E + (n + 1) * 512],
                            start=(k == 0), stop=(k == EK - 1))
                    nc.vector.tensor_add(
                        v_tiles[mb * 4 + i][:, n * 512:(n + 1) * 512],
                        ps[:], bv_bc[:, n * 512:(n + 1) * 512])

        # ======== phase C: QT = Wq^T @ xqT + bq ========
        nc.sync.dma_start(wq_sb[:, 0:2 * E], wqh[:, 0:2 * E])
        nc.sync.dma_start(xq_sb[:, 0], xqh[:, 0])
        nc.sync.dma_start(wq_sb[:, 2 * E:8 * E], wqh[:, 2 * E:8 * E])
        nc.sync.dma_start(xq_sb[:, 1], xqh[:, 1])
        for n in range(SQ // 512):
            for m in range(EK):
                ps = psMain.tile([128, 512], F32, tag="ps")
                for k in range(EK):
                    nc.tensor.matmul(
                        ps[:],
                        wq_sb[:, m * E + k * 128:m * E + (k + 1) * 128],
                        xq_sb[:, n, k],
                        start=(k == 0), stop=(k == EK - 1))
                j, kk = divmod(m, 2)
                nc.vector.tensor_scalar_add(
                    qt_pair[j][:, kk:kk + 1, n * 512:(n + 1) * 512],
                    ps[:], bq_t[:, m:m + 1])

        # ======== phase D: attention, blocked over s_q ========
        for blk in range(NBLK):
            q0 = blk * BQ
            # scoresT = KT^T @ QT_blk; exp (bf16); running sum on DVE
            part = partp.tile([128, BQ], F32, tag="part")
            part_r = partp.tile([128, BQ], F32R, tag="part_r")
            for m in range(MK):
                ps = psMain.tile([128, 512], F32, tag="ps")
                if USE_FP8_SCORES:
                    for j in range(EK // 2):
                        nc.tensor.matmul(
                            ps[:],
                            kt_pair[j][:, 0:2, m * 128:(m + 1) * 128],
                            qt_pair[j][:, 0:2, q0:q0 + BQ],
                            start=(j == 0), stop=(j == EK // 2 - 1),
                            perf_mode=DR)
                else:
                    for j in range(EK // 2):
                        for kk in range(2):
                            nc.tensor.matmul(
                                ps[:],
                                kt_pair[j][:, kk, m * 128:(m + 1) * 128],
                                qt_pair[j][:, kk, q0:q0 + BQ],
                                start=(j == 0 and kk == 0),
                                stop=(j == EK // 2 - 1 and kk == 1))
                nc.scalar.activation(exp_tiles[m][:], ps[:], AF.Exp,
                                     scale=INV_SCALE)
                if m == 1:
                    nc.vector.tensor_add(part[:], exp_tiles[0][:],
                                         exp_tiles[1][:])
                elif m == MK - 1:
                    nc.vector.tensor_add(part_r[:], part[:], exp_tiles[m][:])
                elif m > 1:
                    nc.vector.tensor_add(part[:], part[:], exp_tiles[m][:])

            # out_unnorm[s_q, e] = expT^T @ V ; sums via ones-matmul.
            # The pssum matmul is emitted AFTER the first psO group's
            # matmuls so the PE never waits on the DVE running-sum chain.
            for mi in range(BQ // 128):
                g = blk * (BQ // 128) + mi
                ot = outp.tile([128, E], F32, tag="ot")
                recip = sumsp.tile([128, 1], F32, tag="recip")
                last = (blk == NBLK - 1 and mi == BQ // 128 - 1)
                for n in range(E // 512):
                    pso = psO.tile([128, 512], F32, tag="psO")
                    for m in range(MK):
                        nc.tensor.matmul(
                            pso[:],
                            exp_tiles[m][:, mi * 128:(mi + 1) * 128],
                            v_tiles[m][:, n * 512:(n + 1) * 512],
                            start=(m == 0), stop=(m == MK - 1))
                    if n == 0:
                        pssum = psSum.tile([128, 2], F32, tag="pssum")
                        nc.tensor.matmul(
                            pssum[:],
                            part_r[:, mi * 128:(mi + 1) * 128],
                            ones_t[:], start=True, stop=True)
                        nc.vector.reciprocal(recip[:], pssum[:, 0:1])
                    if last and n == E // 512 - 1:
                        # split the final eviction/DMA to shorten the tail
                        for h2 in range(2):
                            sl = slice(n * 512 + h2 * 256,
                                       n * 512 + (h2 + 1) * 256)
                            nc.vector.tensor_scalar_mul(
                                ot[:, sl], pso[:, h2 * 256:(h2 + 1) * 256],
                                recip[:])
                            nc.sync.dma_start(out[:, g, sl], ot[:, sl])
                    else:
                        nc.vector.tensor_scalar_mul(
                            ot[:, n * 512:(n + 1) * 512], pso[:], recip[:])
                        nc.sync.dma_start(
                            out[:, g, n * 512:(n + 1) * 512],
                            ot[:, n * 512:(n + 1) * 512])

    nc.compile()
    return nc


def _get_nc():
    if "nc" not in _cached:
        _cached["nc"] = _build()
    return _cached["nc"]


def _pack_w_mmajor(W):
    # [128, 8(m), 8(k), 128]: [p, m, k, c] = W[k*128+p, m*128+c]
    return np.ascontiguousarray(
        W.reshape(EK, 128, EK, 128).transpose(1, 2, 0, 3).reshape(128, E * EK)
        .astype(NPBF))


def _pack_w_kmajor(W):
    # [128, 8(k), 1024]: [p, k, c] = W[k*128+p, c]
    return np.ascontiguousarray(
        W.reshape(EK, 128, E).transpose(1, 0, 2).reshape(128, E * EK)
        .astype(NPBF))


def _pack_x(x, blk):
    # [128, nb, 8(k), blk]: [p, nb, k, s] = x[nb*blk+s, k*128+p]
    nb = x.shape[0] // blk
    return np.ascontiguousarray(
        x.reshape(nb, blk, EK, 128).transpose(3, 0, 2, 1).astype(NPBF))


def kernel(query, key, value, Wq, bq, Wk, bk, Wv, bv, **kw):
    query = np.asarray(query, dtype=np.float32)
    key = np.asarray(key, dtype=np.float32)
    value = np.asarray(value, dtype=np.float32)
    wq_h = _pack_w_mmajor(np.asarray(Wq, dtype=np.float32))
    wk_h = _pack_w_mmajor(np.asarray(Wk, dtype=np.float32))
    wv_h = _pack_w_kmajor(np.asarray(Wv, dtype=np.float32))
    bq = np.asarray(bq, dtype=np.float32)
    bk = np.asarray(bk, dtype=np.float32)
    bv = np.asarray(bv, dtype=np.float32)

    cst_h = np.empty((128, 18), dtype=np.float32)
    cst_h[:, 0:2] = 1.0
    cst_h[:, 2:10] = bq.reshape(EK, 128).T
    cst_h[:, 10:18] = bk.reshape(EK, 128).T
    bv_h = np.ascontiguousarray(
        np.broadcast_to(bv.reshape(1, E), (128, E)).astype(NPBF))

    xk_h = {b: _pack_x(key[b], 512) for b in range(B)}
    xv_h = {b: _pack_x(value[b], 512) for b in range(B)}

    in_maps = []
    for c in range(N_CORES):
        b, h = divmod(c, 2)
        xq_h = _pack_x(query[b, h * SQ:(h + 1) * SQ, :], 512)
        in_maps.append({
            "xqh": xq_h, "xkh": xk_h[b], "xvh": xv_h[b],
            "wqh": wq_h, "wkh": wk_h, "wvh": wv_h,
            "csth": cst_h, "bvh": bv_h,
            "ones_in": np.ones((128, 2), dtype=np.float32),
        })

    nc = _get_nc()
    res = bass_utils.run_bass_kernel_spmd(
        nc, in_maps, core_ids=list(range(N_CORES)), **kw)

    full = np.empty((B, S, E), dtype=np.float32)
    for c in range(N_CORES):
        b, h = divmod(c, 2)
        # out [128, 8, E] -> [SQ, E]
        o = res.results[c]["out"]
        full[b, h * SQ:(h + 1) * SQ, :] = o.transpose(1, 0, 2).reshape(SQ, E)
    kernel.last_results = res
    return full
